# revision 68
# baseline (speedup 1.0000x reference)
"""Trainium2 Bass kernel for BodyStructureLoss.

Computes: mean over (B, J) of where(||kps[b,j,:]|| > 1.0, ||kps[b,j,:]||, 0)
for kps of shape [524288, 17, 3] float32.

Strategy (data-parallel over 8 NeuronCores):
  - Each core gets B/8 = 65536 batch rows = 3,342,336 contiguous floats,
    viewed as [128 partitions, 26112] (each partition row holds 8704
    complete (x,y,z) triplets).
  - Tiles of F columns stream in via DMA; squares run in-place (ACT, or DVE
    for a few tiles to balance engines); DVE sums the 3 squared components
    with two strided adds into a shared per-pair s tile; per tile PAIR one
    ACT sqrt and two DVE tensor_scalar+accumulate ops produce
    sum(max(d,1)) and count(s>1) columns.
  - Per core the [128, 2*n_pairs] accumulator tile is DMA'd out directly;
    the host sums all partials across cores and applies
    masked_sum = sum(max(d,1)) + count - B*J, then divides by B*J.
"""

import os

import numpy as np

# the NTFF trace path needs antenv.axon_hooks, which this client image lacks;
# force-disable so a stray BASS_TRACE=1 in the environment cannot break runs
os.environ["BASS_NEVER_TRACE"] = "1"

import concourse.bacc as bacc
import concourse.mybir as mybir
from concourse.bass_utils import run_bass_kernel_spmd
from concourse.tile import TileContext

B, J, D = 524288, 17, 3
HALF_BODY = 1.0  # threshold/2 with threshold=2.0
N_CORES = 8
B_SHARD = B // N_CORES  # 65536
P = 128
FLOATS_PER_CORE = B_SHARD * J * D  # 3342336
COLS = FLOATS_PER_CORE // P  # 26112 (divisible by 3: 26112 = 3*8704)

_DT = mybir.dt.float32

# default plan: pairs of tile column-counts; each pair shares one sqrt.
# small first pair ramps the compute pipeline early; small tail pairs
# shorten the post-DMA drain.
PLAN = [[480, 480]] + [[1632, 1632]] * 7 + [[744, 744], [408, 408]]
DVE_SQ = frozenset({6})  # tile indices squared on DVE instead of ACT
POOL_SQ = frozenset()  # tile indices squared on Pool (gpsimd)
# 1632-tiles whose square is split ACT (front 65%) / Pool (back 35%)
SPLIT_SQ = frozenset(range(2, 16))
# pairs that keep the count+max form (cheap 4x max on the tail spine);
# the host subtracts P*M2 per counted pair
STT_SKIP = frozenset({6, 7, 8, 9})


def build_nc(P=P, COLS=COLS, plan=None, dve_sq=DVE_SQ, pipelined=True, repeat=1, lag=2, flush_from=None, split_sq=SPLIT_SQ, pool_mode="stack", flush_before=None, pool_sq=POOL_SQ, count_pool=False, use_prio=False, wb_out=True, stt=True, add1_pool=frozenset(), stt_skip=STT_SKIP, xbufs=8, split_frac=0.65, adds_pool=frozenset({9}), s_bf16=False, split_frac_tail=None, tail_from=12):
    import bass_rust
    NameSet = bass_rust.InstructionNameOrderedSet
    if plan is None:
        plan = PLAN
    flat = [f for pair in plan for f in pair]
    assert sum(flat) == COLS
    assert all(f % 3 == 0 for f in flat)
    n_pairs = len(plan)
    M_MAX = max(sum(pair) for pair in plan) // 3

    nc = bacc.Bacc(
        "TRN2", target_bir_lowering=False, debug=False, num_devices=N_CORES
    )
    # stt pairs use one accumulator column; stt_skip pairs use two
    # (sum(max(d,1)) and count), requiring the host-side -M2*P correction
    col_of = {}
    off = 0
    for r in range(repeat):
        for pi0 in range(n_pairs):
            col_of[r * n_pairs + pi0] = off
            off += 2 if (not stt or pi0 in stt_skip) else 1
    acc_cols = off
    x = nc.dram_tensor("x", [P, COLS], _DT, kind="ExternalInput")
    out = nc.dram_tensor("out", [P, acc_cols], _DT, kind="ExternalOutput")

    with TileContext(nc, pool_alloc_mode=pool_mode) as tc:
        with (
            tc.tile_pool(name="xin", bufs=xbufs) as xpool,
            tc.tile_pool(name="small", bufs=4) as spool,
            tc.tile_pool(name="accp", bufs=1) as accpool,
        ):
            # stt: one accumulator column per pair, sum(d * (d > 1));
            # else two: sum(max(d,1)) and count(s>1)
            accs = accpool.tile([P, acc_cols], _DT)
            # shared scratch for tensor_scalar main outputs (only accum_out
            # is consumed); WAW chains are DVE-internal and in-order.
            # scr32 is only read by count pairs, so size it to their max M2.
            m32 = max([sum(plan[pi]) // 3 for pi in stt_skip] or [1]) if stt else M_MAX
            scr32 = accpool.tile([P, max(m32, 2)], _DT)
            scr16 = accpool.tile([P, M_MAX], mybir.dt.bfloat16)

            # dummy sqrt first: makes bacc's table pass load sqrt_and_others
            # (which also contains Square), avoiding a second ACT table load
            nc.vector.memset(scr32[:, :1], 1.0)
            nc.scalar.activation(
                out=scr32[:, :1],
                in_=scr32[:, :1],
                func=mybir.ActivationFunctionType.Sqrt,
            )

            acc_writers = []  # TS instructions writing accs (gate the out-DMA)
            if wb_out:
                wb_idx = accpool.tile([P, 1], mybir.dt.int32)
                nc.gpsimd.memset(wb_idx, 0)
                wb_sem = nc.alloc_semaphore("wb_dma")

            # scheduler priority: the tile scheduler pops the lowest
            # bass_priority READY instruction per engine; strict pair-order
            # priorities make a ready stage_b op always outrank later pairs'
            # squares on the same engine.
            def prio(ret, pi, off):
                if use_prio:
                    ret.ins.bass_priority = pi * 100 + off
                return ret

            # stage A (per tile): DMA -> square in place -> adds into s slice
            def stage_a(gi, pi, col0, F, s2, s_off):
                M = F // 3
                sz = str(F)
                nb = xbufs if F < 2448 else min(xbufs, 3)
                xb = {"bufs": nb} if nb != xbufs else {}
                xt = xpool.tile([P, F], _DT, tag="xt" + sz, **xb)
                prio(nc.sync.dma_start(out=xt, in_=x[:, col0 : col0 + F]), pi, 1)

                if gi in dve_sq:
                    r = nc.vector.tensor_tensor(
                        out=xt, in0=xt, in1=xt, op=mybir.AluOpType.mult
                    )
                elif gi in pool_sq:
                    r = nc.gpsimd.tensor_tensor(
                        out=xt, in0=xt, in1=xt, op=mybir.AluOpType.mult
                    )
                elif gi in split_sq:
                    # fractional engine split: ACT squares the front part,
                    # Pool (gpsimd) the back part of this tile. Chunks stay
                    # well under the pair cadence, so Pool cannot convoy.
                    f = split_frac if (split_frac_tail is None or gi < tail_from) else split_frac_tail
                    c = int(F * f) & ~3
                    prio(nc.scalar.activation(
                        out=xt[:, :c], in_=xt[:, :c],
                        func=mybir.ActivationFunctionType.Square,
                    ), pi, 2)
                    r = nc.gpsimd.tensor_tensor(
                        out=xt[:, c:], in0=xt[:, c:], in1=xt[:, c:],
                        op=mybir.AluOpType.mult,
                    )
                else:
                    r = nc.scalar.activation(
                        out=xt, in_=xt, func=mybir.ActivationFunctionType.Square
                    )
                prio(r, pi, 2)
                sq3 = xt.rearrange("p (m t) -> p m t", t=3)
                sl = s2[:, s_off : s_off + M]
                # adds_pool pairs run both adds on Pool: at the tail this
                # takes the s-computation off the (saturated) DVE chain
                eng1 = nc.gpsimd if pi in (add1_pool | adds_pool) else nc.vector
                eng2 = nc.gpsimd if pi in adds_pool else nc.vector
                prio(eng1.tensor_tensor(
                    out=sl, in0=sq3[:, :, 0], in1=sq3[:, :, 1], op=mybir.AluOpType.add
                ), pi, 3)
                prio(eng2.tensor_tensor(
                    out=sl, in0=sl, in1=sq3[:, :, 2], op=mybir.AluOpType.add
                ), pi, 4)

            # stage B (per pair): ACT sqrt -> DVE masked accumulate
            def stage_b(pi, s2, M2):
                sz = str(M2)
                c0 = col_of[pi]
                pi0 = pi % n_pairs
                use_stt = stt and pi0 not in stt_skip
                if not use_stt:
                    # count(s > 1): only depends on s, runs while ACT sqrts.
                    # With bf16 s the all-bf16 operands put this TS in DVE 4x
                    # mode. (TensorScalarPtr is DVE-only on core V3 —
                    # count_pool fails walrus codegen; sim experiments only.)
                    ceng = nc.gpsimd if count_pool else nc.vector
                    cscr = scr16 if s2.dtype == mybir.dt.bfloat16 else scr32
                    acc_writers.append(prio(ceng.tensor_scalar(
                        out=cscr[:, :M2],
                        in0=s2,
                        scalar1=float(HALF_BODY * HALF_BODY),
                        scalar2=None,
                        op0=mybir.AluOpType.is_gt,
                        op1=mybir.AluOpType.add,
                        accum_out=accs[:, c0 + 1 : c0 + 2],
                    ), pi, 5))
                db = {"bufs": 2} if M2 >= 2176 else {}
                d = spool.tile([P, M2], mybir.dt.bfloat16, tag="d" + sz, **db)
                prio(nc.scalar.activation(
                    out=d, in_=s2, func=mybir.ActivationFunctionType.Sqrt
                ), pi, 6)
                if use_stt:
                    # one fused DVE op: accum += sum((s > 1) * d). The mask
                    # comes from fp32 s (exact), so bf16 d contributes only
                    # symmetric per-element rounding noise that cancels.
                    acc_writers.append(prio(nc.vector.scalar_tensor_tensor(
                        out=scr16[:, :M2],
                        in0=s2,
                        scalar=float(HALF_BODY * HALF_BODY),
                        in1=d,
                        op0=mybir.AluOpType.is_gt,
                        op1=mybir.AluOpType.mult,
                        accum_out=accs[:, c0 : c0 + 1],
                    ), pi, 7))
                else:
                    # sum(max(d, 1)) = masked_sum + M2 - count per partition;
                    # bf16 all-operands makes this TS 4x mode (cheap on the
                    # tail spine). Host subtracts P*M2 for these pairs.
                    acc_writers.append(prio(nc.vector.tensor_scalar(
                        out=scr16[:, :M2],
                        in0=d,
                        scalar1=float(HALF_BODY),
                        scalar2=None,
                        op0=mybir.AluOpType.max,
                        op1=mybir.AluOpType.add,
                        accum_out=accs[:, c0 : c0 + 1],
                    ), pi, 7))

            # emit: stage_a per tile; stage_b lags `lag` pairs behind.
            # repeat>1 re-runs the whole pass (benchmarking only).
            from collections import deque

            pending = deque()  # (pair_idx, s2, M2)
            for r in range(repeat):
                col0 = 0
                gi = 0
                for pi0, pair in enumerate(plan):
                    pi = r * n_pairs + pi0
                    # emit pending stage_b BEFORE this pair's stage_a, so a
                    # ready sqrt is not queued behind a DMA-gated square
                    if flush_before is not None and pi0 >= flush_before:
                        while pending:
                            stage_b(*pending.popleft())
                    M2 = sum(pair) // 3
                    sb = {"bufs": 2} if M2 >= 2176 else {}
                    # bf16 s for count-style pairs: count/max both hit DVE 4x.
                    # Cost: the bf16 band just above s=1 is excluded from both
                    # count and the d>1 region (~1e-4 systematic, gate 2e-2).
                    sdt = (mybir.dt.bfloat16
                           if (s_bf16 and stt and pi0 in stt_skip) else _DT)
                    stag = ("sb" if sdt == mybir.dt.bfloat16 else "s") + str(M2)
                    s2 = spool.tile([P, M2], sdt, tag=stag, **sb)
                    s_off = 0
                    for F in pair:
                        stage_a(gi, pi, col0, F, s2, s_off)
                        col0 += F
                        s_off += F // 3
                        gi += 1
                    if pipelined:
                        pending.append((pi, s2, M2))
                        # from pair `flush_from` on, emit stage_b immediately
                        # so tail sqrts outrank later squares in the scheduler
                        eff_lag = 0 if (flush_from is not None and pi0 >= flush_from) else lag
                        while len(pending) > eff_lag:
                            stage_b(*pending.popleft())
                    else:
                        stage_b(pi, s2, M2)
            while pending:
                stage_b(*pending.popleft())

            if not wb_out:
                nc.sync.dma_start(out=out[:, :], in_=accs)
                wb_prep = None
            else:
                # out-DMA via pre-staged SWDGE descriptors: the prep only
                # writes descriptors (no HWDGE/dge-delay on the tail), the
                # trigger fires the tiny transfer after the last accum.
                in_view = accs[:, :].rearrange("p (a b w) -> p a b w", a=1, b=1)
                out_view = out[:, :].rearrange("p (a b w) -> a p b w", a=1, b=1)
                wb_prep = nc.gpsimd.kv_writeback(
                    out_view, in_view, wb_idx[:, :], prepare_only=True,
                    sem=wb_sem,
                )
                wb_trig = nc.gpsimd.trigger_dma(count=None)
                # defer the RAW edges on accs from the prep to the trigger
                # (what bass_rust's swdge_deferred_ins does for scatter_add;
                # kv_writeback is not in that table)
                acc_names = {w.ins.name for w in acc_writers}
                prep_sync = list(wb_prep.ins.sync_dependency_names())
                wb_prep.ins.set_sync_dependencies(
                    NameSet([d for d in prep_sync if d not in acc_names])
                )
                # no nosync edges either: the prep writes descriptors only,
                # so it carries no ordering against the accum writers at all
                wb_prep.ins.set_nosync_dependencies(NameSet(
                    [d for d in wb_prep.ins.nosync_dependency_names()
                     if d not in acc_names]
                ))
                wb_trig.ins.set_sync_dependencies(NameSet(
                    list(wb_trig.ins.sync_dependency_names())
                    + sorted(acc_names)
                ))

    nc.compile()

    if wb_prep is not None:
        # the cost model fires the prep's on_update[0] at trigger time, and
        # the teardown drain waits on the framework's DMASW queue sem; point
        # on_update[0] at that sem (scatter_add gets this wiring natively)
        dmasw = None
        for i in nc.all_instructions():
            if i.sync_info:
                for w in i.sync_info.on_wait:
                    if w.ant_name and w.ant_name.startswith("DMASW"):
                        dmasw = (w.id, w.ant_name)
        assert dmasw is not None, "no DMASW drain wait found"
        wb_prep.ins.sync_info.on_update[0] = mybir.SyncUpdate(
            sync_type="semaphore", id=dmasw[0], ant_name=dmasw[1],
            update_mode="sem-add-imm", update_value=16,
        )
    return nc


_nc_cache = None
last_results = None


def kernel(kps_world_pred: np.ndarray) -> np.ndarray:
    global _nc_cache, last_results
    x = np.ascontiguousarray(kps_world_pred, dtype=np.float32)
    assert x.shape == (B, J, D)

    shards = x.reshape(N_CORES, P, COLS)
    in_maps = [{"x": shards[c]} for c in range(N_CORES)]

    if _nc_cache is None:
        _nc_cache = build_nc()

    # the axon terminal occasionally reports a transient
    # NRT_EXEC_UNIT_UNRECOVERABLE left over from a previous run; it clears
    # after a short wait, so retry rather than fail the whole call
    import time

    res = None
    for attempt in range(3):
        try:
            res = run_bass_kernel_spmd(_nc_cache, in_maps, list(range(N_CORES)))
            break
        except Exception:
            if attempt == 2:
                raise
            time.sleep(15)
    last_results = res

    # stt pairs contribute sum(d * (d > 1)) directly; stt_skip pairs
    # contribute sum(max(d,1)) + count = masked_sum + P*M2 per core
    total = np.float64(0.0)
    for c in range(N_CORES):
        total += res.results[c]["out"].astype(np.float64).sum()
    skip_m2 = sum(sum(PLAN[pi]) // 3 for pi in STT_SKIP)
    total -= np.float64(N_CORES * P * skip_m2)
    return np.asarray(total / (B * J), dtype=np.float32)



# revision 70
# speedup vs baseline: 1.0017x; 1.0017x over previous
"""Trainium2 Bass kernel for BodyStructureLoss.

Computes: mean over (B, J) of where(||kps[b,j,:]|| > 1.0, ||kps[b,j,:]||, 0)
for kps of shape [524288, 17, 3] float32.

Strategy (data-parallel over 8 NeuronCores):
  - Each core gets B/8 = 65536 batch rows = 3,342,336 contiguous floats,
    viewed as [128 partitions, 26112] (each partition row holds 8704
    complete (x,y,z) triplets).
  - Tiles of F columns stream in via DMA; squares run in-place (ACT, or DVE
    for a few tiles to balance engines); DVE sums the 3 squared components
    with two strided adds into a shared per-pair s tile; per tile PAIR one
    ACT sqrt and two DVE tensor_scalar+accumulate ops produce
    sum(max(d,1)) and count(s>1) columns.
  - Per core the [128, 2*n_pairs] accumulator tile is DMA'd out directly;
    the host sums all partials across cores and applies
    masked_sum = sum(max(d,1)) + count - B*J, then divides by B*J.
"""

import os

import numpy as np

# the NTFF trace path needs antenv.axon_hooks, which this client image lacks;
# force-disable so a stray BASS_TRACE=1 in the environment cannot break runs
os.environ["BASS_NEVER_TRACE"] = "1"

import concourse.bacc as bacc
import concourse.mybir as mybir
from concourse.bass_utils import run_bass_kernel_spmd
from concourse.tile import TileContext

B, J, D = 524288, 17, 3
HALF_BODY = 1.0  # threshold/2 with threshold=2.0
N_CORES = 8
B_SHARD = B // N_CORES  # 65536
P = 128
FLOATS_PER_CORE = B_SHARD * J * D  # 3342336
COLS = FLOATS_PER_CORE // P  # 26112 (divisible by 3: 26112 = 3*8704)

_DT = mybir.dt.float32

# default plan: pairs of tile column-counts; each pair shares one sqrt.
# small first pair ramps the compute pipeline early; small tail pairs
# shorten the post-DMA drain.
PLAN = [[480, 480]] + [[1632, 1632]] * 7 + [[744, 744], [408, 408]]
DVE_SQ = frozenset({9})  # tile indices squared on DVE instead of ACT
POOL_SQ = frozenset()  # tile indices squared on Pool (gpsimd)
# 1632-tiles whose square is split ACT (front 65%) / Pool (back 35%)
SPLIT_SQ = frozenset(range(2, 16))
# pairs that keep the count+max form (cheap 4x max on the tail spine);
# the host subtracts P*M2 per counted pair
STT_SKIP = frozenset({6, 7, 8, 9})


def build_nc(P=P, COLS=COLS, plan=None, dve_sq=DVE_SQ, pipelined=True, repeat=1, lag=2, flush_from=None, split_sq=SPLIT_SQ, pool_mode="stack", flush_before=None, pool_sq=POOL_SQ, count_pool=False, use_prio=False, wb_out=True, stt=True, add1_pool=frozenset(), stt_skip=STT_SKIP, xbufs=6, split_frac=0.65, adds_pool=frozenset({9}), s_bf16=False, split_frac_tail=None, tail_from=12):
    import bass_rust
    NameSet = bass_rust.InstructionNameOrderedSet
    if plan is None:
        plan = PLAN
    flat = [f for pair in plan for f in pair]
    assert sum(flat) == COLS
    assert all(f % 3 == 0 for f in flat)
    n_pairs = len(plan)
    M_MAX = max(sum(pair) for pair in plan) // 3

    nc = bacc.Bacc(
        "TRN2", target_bir_lowering=False, debug=False, num_devices=N_CORES
    )
    # stt pairs use one accumulator column; stt_skip pairs use two
    # (sum(max(d,1)) and count), requiring the host-side -M2*P correction
    col_of = {}
    off = 0
    for r in range(repeat):
        for pi0 in range(n_pairs):
            col_of[r * n_pairs + pi0] = off
            off += 2 if (not stt or pi0 in stt_skip) else 1
    acc_cols = off
    x = nc.dram_tensor("x", [P, COLS], _DT, kind="ExternalInput")
    out = nc.dram_tensor("out", [P, acc_cols], _DT, kind="ExternalOutput")

    with TileContext(nc, pool_alloc_mode=pool_mode) as tc:
        with (
            tc.tile_pool(name="xin", bufs=xbufs) as xpool,
            tc.tile_pool(name="small", bufs=4) as spool,
            tc.tile_pool(name="accp", bufs=1) as accpool,
        ):
            # stt: one accumulator column per pair, sum(d * (d > 1));
            # else two: sum(max(d,1)) and count(s>1)
            accs = accpool.tile([P, acc_cols], _DT)
            # shared scratch for tensor_scalar main outputs (only accum_out
            # is consumed); WAW chains are DVE-internal and in-order.
            # scr32 is only read by count pairs, so size it to their max M2.
            m32 = max([sum(plan[pi]) // 3 for pi in stt_skip] or [1]) if stt else M_MAX
            scr32 = accpool.tile([P, max(m32, 2)], _DT)
            scr16 = accpool.tile([P, M_MAX], mybir.dt.bfloat16)

            # dummy sqrt first: makes bacc's table pass load sqrt_and_others
            # (which also contains Square), avoiding a second ACT table load
            nc.vector.memset(scr32[:, :1], 1.0)
            nc.scalar.activation(
                out=scr32[:, :1],
                in_=scr32[:, :1],
                func=mybir.ActivationFunctionType.Sqrt,
            )

            acc_writers = []  # TS instructions writing accs (gate the out-DMA)
            if wb_out:
                wb_idx = accpool.tile([P, 1], mybir.dt.int32)
                nc.gpsimd.memset(wb_idx, 0)
                wb_sem = nc.alloc_semaphore("wb_dma")

            # scheduler priority: the tile scheduler pops the lowest
            # bass_priority READY instruction per engine; strict pair-order
            # priorities make a ready stage_b op always outrank later pairs'
            # squares on the same engine.
            def prio(ret, pi, off):
                if use_prio:
                    ret.ins.bass_priority = pi * 100 + off
                return ret

            # stage A (per tile): DMA -> square in place -> adds into s slice
            def stage_a(gi, pi, col0, F, s2, s_off):
                M = F // 3
                sz = str(F)
                nb = xbufs if F < 2448 else min(xbufs, 3)
                xb = {"bufs": nb} if nb != xbufs else {}
                xt = xpool.tile([P, F], _DT, tag="xt" + sz, **xb)
                prio(nc.sync.dma_start(out=xt, in_=x[:, col0 : col0 + F]), pi, 1)

                if gi in dve_sq:
                    r = nc.vector.tensor_tensor(
                        out=xt, in0=xt, in1=xt, op=mybir.AluOpType.mult
                    )
                elif gi in pool_sq:
                    r = nc.gpsimd.tensor_tensor(
                        out=xt, in0=xt, in1=xt, op=mybir.AluOpType.mult
                    )
                elif gi in split_sq:
                    # fractional engine split: ACT squares the front part,
                    # Pool (gpsimd) the back part of this tile. Chunks stay
                    # well under the pair cadence, so Pool cannot convoy.
                    f = split_frac if (split_frac_tail is None or gi < tail_from) else split_frac_tail
                    c = int(F * f) & ~3
                    prio(nc.scalar.activation(
                        out=xt[:, :c], in_=xt[:, :c],
                        func=mybir.ActivationFunctionType.Square,
                    ), pi, 2)
                    r = nc.gpsimd.tensor_tensor(
                        out=xt[:, c:], in0=xt[:, c:], in1=xt[:, c:],
                        op=mybir.AluOpType.mult,
                    )
                else:
                    r = nc.scalar.activation(
                        out=xt, in_=xt, func=mybir.ActivationFunctionType.Square
                    )
                prio(r, pi, 2)
                sq3 = xt.rearrange("p (m t) -> p m t", t=3)
                sl = s2[:, s_off : s_off + M]
                # adds_pool pairs run both adds on Pool: at the tail this
                # takes the s-computation off the (saturated) DVE chain
                eng1 = nc.gpsimd if pi in (add1_pool | adds_pool) else nc.vector
                eng2 = nc.gpsimd if pi in adds_pool else nc.vector
                prio(eng1.tensor_tensor(
                    out=sl, in0=sq3[:, :, 0], in1=sq3[:, :, 1], op=mybir.AluOpType.add
                ), pi, 3)
                prio(eng2.tensor_tensor(
                    out=sl, in0=sl, in1=sq3[:, :, 2], op=mybir.AluOpType.add
                ), pi, 4)

            # stage B (per pair): ACT sqrt -> DVE masked accumulate
            def stage_b(pi, s2, M2):
                sz = str(M2)
                c0 = col_of[pi]
                pi0 = pi % n_pairs
                use_stt = stt and pi0 not in stt_skip
                if not use_stt:
                    # count(s > 1): only depends on s, runs while ACT sqrts.
                    # With bf16 s the all-bf16 operands put this TS in DVE 4x
                    # mode. (TensorScalarPtr is DVE-only on core V3 —
                    # count_pool fails walrus codegen; sim experiments only.)
                    ceng = nc.gpsimd if count_pool else nc.vector
                    cscr = scr16 if s2.dtype == mybir.dt.bfloat16 else scr32
                    acc_writers.append(prio(ceng.tensor_scalar(
                        out=cscr[:, :M2],
                        in0=s2,
                        scalar1=float(HALF_BODY * HALF_BODY),
                        scalar2=None,
                        op0=mybir.AluOpType.is_gt,
                        op1=mybir.AluOpType.add,
                        accum_out=accs[:, c0 + 1 : c0 + 2],
                    ), pi, 5))
                db = {"bufs": 2} if M2 >= 2176 else {}
                d = spool.tile([P, M2], mybir.dt.bfloat16, tag="d" + sz, **db)
                prio(nc.scalar.activation(
                    out=d, in_=s2, func=mybir.ActivationFunctionType.Sqrt
                ), pi, 6)
                if use_stt:
                    # one fused DVE op: accum += sum((s > 1) * d). The mask
                    # comes from fp32 s (exact), so bf16 d contributes only
                    # symmetric per-element rounding noise that cancels.
                    acc_writers.append(prio(nc.vector.scalar_tensor_tensor(
                        out=scr16[:, :M2],
                        in0=s2,
                        scalar=float(HALF_BODY * HALF_BODY),
                        in1=d,
                        op0=mybir.AluOpType.is_gt,
                        op1=mybir.AluOpType.mult,
                        accum_out=accs[:, c0 : c0 + 1],
                    ), pi, 7))
                else:
                    # sum(max(d, 1)) = masked_sum + M2 - count per partition;
                    # bf16 all-operands makes this TS 4x mode (cheap on the
                    # tail spine). Host subtracts P*M2 for these pairs.
                    acc_writers.append(prio(nc.vector.tensor_scalar(
                        out=scr16[:, :M2],
                        in0=d,
                        scalar1=float(HALF_BODY),
                        scalar2=None,
                        op0=mybir.AluOpType.max,
                        op1=mybir.AluOpType.add,
                        accum_out=accs[:, c0 : c0 + 1],
                    ), pi, 7))

            # emit: stage_a per tile; stage_b lags `lag` pairs behind.
            # repeat>1 re-runs the whole pass (benchmarking only).
            from collections import deque

            pending = deque()  # (pair_idx, s2, M2)
            for r in range(repeat):
                col0 = 0
                gi = 0
                for pi0, pair in enumerate(plan):
                    pi = r * n_pairs + pi0
                    # emit pending stage_b BEFORE this pair's stage_a, so a
                    # ready sqrt is not queued behind a DMA-gated square
                    if flush_before is not None and pi0 >= flush_before:
                        while pending:
                            stage_b(*pending.popleft())
                    M2 = sum(pair) // 3
                    sb = {"bufs": 2} if M2 >= 2176 else {}
                    # bf16 s for count-style pairs: count/max both hit DVE 4x.
                    # Cost: the bf16 band just above s=1 is excluded from both
                    # count and the d>1 region (~1e-4 systematic, gate 2e-2).
                    sdt = (mybir.dt.bfloat16
                           if (s_bf16 and stt and pi0 in stt_skip) else _DT)
                    stag = ("sb" if sdt == mybir.dt.bfloat16 else "s") + str(M2)
                    s2 = spool.tile([P, M2], sdt, tag=stag, **sb)
                    s_off = 0
                    for F in pair:
                        stage_a(gi, pi, col0, F, s2, s_off)
                        col0 += F
                        s_off += F // 3
                        gi += 1
                    if pipelined:
                        pending.append((pi, s2, M2))
                        # from pair `flush_from` on, emit stage_b immediately
                        # so tail sqrts outrank later squares in the scheduler
                        eff_lag = 0 if (flush_from is not None and pi0 >= flush_from) else lag
                        while len(pending) > eff_lag:
                            stage_b(*pending.popleft())
                    else:
                        stage_b(pi, s2, M2)
            while pending:
                stage_b(*pending.popleft())

            if not wb_out:
                nc.sync.dma_start(out=out[:, :], in_=accs)
                wb_prep = None
            else:
                # out-DMA via pre-staged SWDGE descriptors: the prep only
                # writes descriptors (no HWDGE/dge-delay on the tail), the
                # trigger fires the tiny transfer after the last accum.
                in_view = accs[:, :].rearrange("p (a b w) -> p a b w", a=1, b=1)
                out_view = out[:, :].rearrange("p (a b w) -> a p b w", a=1, b=1)
                wb_prep = nc.gpsimd.kv_writeback(
                    out_view, in_view, wb_idx[:, :], prepare_only=True,
                    sem=wb_sem,
                )
                wb_trig = nc.gpsimd.trigger_dma(count=None)
                # defer the RAW edges on accs from the prep to the trigger
                # (what bass_rust's swdge_deferred_ins does for scatter_add;
                # kv_writeback is not in that table)
                acc_names = {w.ins.name for w in acc_writers}
                prep_sync = list(wb_prep.ins.sync_dependency_names())
                wb_prep.ins.set_sync_dependencies(
                    NameSet([d for d in prep_sync if d not in acc_names])
                )
                # no nosync edges either: the prep writes descriptors only,
                # so it carries no ordering against the accum writers at all
                wb_prep.ins.set_nosync_dependencies(NameSet(
                    [d for d in wb_prep.ins.nosync_dependency_names()
                     if d not in acc_names]
                ))
                wb_trig.ins.set_sync_dependencies(NameSet(
                    list(wb_trig.ins.sync_dependency_names())
                    + sorted(acc_names)
                ))

    nc.compile()

    if wb_prep is not None:
        # the cost model fires the prep's on_update[0] at trigger time, and
        # the teardown drain waits on the framework's DMASW queue sem; point
        # on_update[0] at that sem (scatter_add gets this wiring natively)
        dmasw = None
        for i in nc.all_instructions():
            if i.sync_info:
                for w in i.sync_info.on_wait:
                    if w.ant_name and w.ant_name.startswith("DMASW"):
                        dmasw = (w.id, w.ant_name)
        assert dmasw is not None, "no DMASW drain wait found"
        wb_prep.ins.sync_info.on_update[0] = mybir.SyncUpdate(
            sync_type="semaphore", id=dmasw[0], ant_name=dmasw[1],
            update_mode="sem-add-imm", update_value=16,
        )
    return nc


_nc_cache = None
last_results = None


def kernel(kps_world_pred: np.ndarray) -> np.ndarray:
    global _nc_cache, last_results
    x = np.ascontiguousarray(kps_world_pred, dtype=np.float32)
    assert x.shape == (B, J, D)

    shards = x.reshape(N_CORES, P, COLS)
    in_maps = [{"x": shards[c]} for c in range(N_CORES)]

    if _nc_cache is None:
        _nc_cache = build_nc()

    # the axon terminal occasionally reports a transient
    # NRT_EXEC_UNIT_UNRECOVERABLE left over from a previous run; it clears
    # after a short wait, so retry rather than fail the whole call
    import time

    res = None
    for attempt in range(3):
        try:
            res = run_bass_kernel_spmd(_nc_cache, in_maps, list(range(N_CORES)))
            break
        except Exception:
            if attempt == 2:
                raise
            time.sleep(15)
    last_results = res

    # stt pairs contribute sum(d * (d > 1)) directly; stt_skip pairs
    # contribute sum(max(d,1)) + count = masked_sum + P*M2 per core
    total = np.float64(0.0)
    for c in range(N_CORES):
        total += res.results[c]["out"].astype(np.float64).sum()
    skip_m2 = sum(sum(PLAN[pi]) // 3 for pi in STT_SKIP)
    total -= np.float64(N_CORES * P * skip_m2)
    return np.asarray(total / (B * J), dtype=np.float32)



# revision 71
# speedup vs baseline: 1.0018x; 1.0001x over previous
"""Trainium2 Bass kernel for BodyStructureLoss.

Computes: mean over (B, J) of where(||kps[b,j,:]|| > 1.0, ||kps[b,j,:]||, 0)
for kps of shape [524288, 17, 3] float32.

Strategy (data-parallel over 8 NeuronCores):
  - Each core gets B/8 = 65536 batch rows = 3,342,336 contiguous floats,
    viewed as [128 partitions, 26112] (each partition row holds 8704
    complete (x,y,z) triplets).
  - Tiles of F columns stream in via DMA; squares run in-place (ACT, or DVE
    for a few tiles to balance engines); DVE sums the 3 squared components
    with two strided adds into a shared per-pair s tile; per tile PAIR one
    ACT sqrt and two DVE tensor_scalar+accumulate ops produce
    sum(max(d,1)) and count(s>1) columns.
  - Per core the [128, 2*n_pairs] accumulator tile is DMA'd out directly;
    the host sums all partials across cores and applies
    masked_sum = sum(max(d,1)) + count - B*J, then divides by B*J.
"""

import os

import numpy as np

# the NTFF trace path needs antenv.axon_hooks, which this client image lacks;
# force-disable so a stray BASS_TRACE=1 in the environment cannot break runs
os.environ["BASS_NEVER_TRACE"] = "1"

import concourse.bacc as bacc
import concourse.mybir as mybir
from concourse.bass_utils import run_bass_kernel_spmd
from concourse.tile import TileContext

B, J, D = 524288, 17, 3
HALF_BODY = 1.0  # threshold/2 with threshold=2.0
N_CORES = 8
B_SHARD = B // N_CORES  # 65536
P = 128
FLOATS_PER_CORE = B_SHARD * J * D  # 3342336
COLS = FLOATS_PER_CORE // P  # 26112 (divisible by 3: 26112 = 3*8704)

_DT = mybir.dt.float32

# default plan: pairs of tile column-counts; each pair shares one sqrt.
# small first pair ramps the compute pipeline early; small tail pairs
# shorten the post-DMA drain.
PLAN = [[456, 456]] + [[1632, 1632]] * 7 + [[768, 768], [408, 408]]
DVE_SQ = frozenset({9})  # tile indices squared on DVE instead of ACT
POOL_SQ = frozenset()  # tile indices squared on Pool (gpsimd)
# 1632-tiles whose square is split ACT (front 65%) / Pool (back 35%)
SPLIT_SQ = frozenset(range(2, 16))
# pairs that keep the count+max form (cheap 4x max on the tail spine);
# the host subtracts P*M2 per counted pair
STT_SKIP = frozenset({6, 7, 8, 9})


def build_nc(P=P, COLS=COLS, plan=None, dve_sq=DVE_SQ, pipelined=True, repeat=1, lag=2, flush_from=None, split_sq=SPLIT_SQ, pool_mode="stack", flush_before=None, pool_sq=POOL_SQ, count_pool=False, use_prio=False, wb_out=True, stt=True, add1_pool=frozenset(), stt_skip=STT_SKIP, xbufs=6, split_frac=0.65, adds_pool=frozenset({9}), s_bf16=False, split_frac_tail=None, tail_from=12):
    import bass_rust
    NameSet = bass_rust.InstructionNameOrderedSet
    if plan is None:
        plan = PLAN
    flat = [f for pair in plan for f in pair]
    assert sum(flat) == COLS
    assert all(f % 3 == 0 for f in flat)
    n_pairs = len(plan)
    M_MAX = max(sum(pair) for pair in plan) // 3

    nc = bacc.Bacc(
        "TRN2", target_bir_lowering=False, debug=False, num_devices=N_CORES
    )
    # stt pairs use one accumulator column; stt_skip pairs use two
    # (sum(max(d,1)) and count), requiring the host-side -M2*P correction
    col_of = {}
    off = 0
    for r in range(repeat):
        for pi0 in range(n_pairs):
            col_of[r * n_pairs + pi0] = off
            off += 2 if (not stt or pi0 in stt_skip) else 1
    acc_cols = off
    x = nc.dram_tensor("x", [P, COLS], _DT, kind="ExternalInput")
    out = nc.dram_tensor("out", [P, acc_cols], _DT, kind="ExternalOutput")

    with TileContext(nc, pool_alloc_mode=pool_mode) as tc:
        with (
            tc.tile_pool(name="xin", bufs=xbufs) as xpool,
            tc.tile_pool(name="small", bufs=4) as spool,
            tc.tile_pool(name="accp", bufs=1) as accpool,
        ):
            # stt: one accumulator column per pair, sum(d * (d > 1));
            # else two: sum(max(d,1)) and count(s>1)
            accs = accpool.tile([P, acc_cols], _DT)
            # shared scratch for tensor_scalar main outputs (only accum_out
            # is consumed); WAW chains are DVE-internal and in-order.
            # scr32 is only read by count pairs, so size it to their max M2.
            m32 = max([sum(plan[pi]) // 3 for pi in stt_skip] or [1]) if stt else M_MAX
            scr32 = accpool.tile([P, max(m32, 2)], _DT)
            scr16 = accpool.tile([P, M_MAX], mybir.dt.bfloat16)

            # dummy sqrt first: makes bacc's table pass load sqrt_and_others
            # (which also contains Square), avoiding a second ACT table load
            nc.vector.memset(scr32[:, :1], 1.0)
            nc.scalar.activation(
                out=scr32[:, :1],
                in_=scr32[:, :1],
                func=mybir.ActivationFunctionType.Sqrt,
            )

            acc_writers = []  # TS instructions writing accs (gate the out-DMA)
            if wb_out:
                wb_idx = accpool.tile([P, 1], mybir.dt.int32)
                nc.gpsimd.memset(wb_idx, 0)
                wb_sem = nc.alloc_semaphore("wb_dma")

            # scheduler priority: the tile scheduler pops the lowest
            # bass_priority READY instruction per engine; strict pair-order
            # priorities make a ready stage_b op always outrank later pairs'
            # squares on the same engine.
            def prio(ret, pi, off):
                if use_prio:
                    ret.ins.bass_priority = pi * 100 + off
                return ret

            # stage A (per tile): DMA -> square in place -> adds into s slice
            def stage_a(gi, pi, col0, F, s2, s_off):
                M = F // 3
                sz = str(F)
                nb = xbufs if F < 2448 else min(xbufs, 3)
                xb = {"bufs": nb} if nb != xbufs else {}
                xt = xpool.tile([P, F], _DT, tag="xt" + sz, **xb)
                prio(nc.sync.dma_start(out=xt, in_=x[:, col0 : col0 + F]), pi, 1)

                if gi in dve_sq:
                    r = nc.vector.tensor_tensor(
                        out=xt, in0=xt, in1=xt, op=mybir.AluOpType.mult
                    )
                elif gi in pool_sq:
                    r = nc.gpsimd.tensor_tensor(
                        out=xt, in0=xt, in1=xt, op=mybir.AluOpType.mult
                    )
                elif gi in split_sq:
                    # fractional engine split: ACT squares the front part,
                    # Pool (gpsimd) the back part of this tile. Chunks stay
                    # well under the pair cadence, so Pool cannot convoy.
                    f = split_frac if (split_frac_tail is None or gi < tail_from) else split_frac_tail
                    c = int(F * f) & ~3
                    prio(nc.scalar.activation(
                        out=xt[:, :c], in_=xt[:, :c],
                        func=mybir.ActivationFunctionType.Square,
                    ), pi, 2)
                    r = nc.gpsimd.tensor_tensor(
                        out=xt[:, c:], in0=xt[:, c:], in1=xt[:, c:],
                        op=mybir.AluOpType.mult,
                    )
                else:
                    r = nc.scalar.activation(
                        out=xt, in_=xt, func=mybir.ActivationFunctionType.Square
                    )
                prio(r, pi, 2)
                sq3 = xt.rearrange("p (m t) -> p m t", t=3)
                sl = s2[:, s_off : s_off + M]
                # adds_pool pairs run both adds on Pool: at the tail this
                # takes the s-computation off the (saturated) DVE chain
                eng1 = nc.gpsimd if pi in (add1_pool | adds_pool) else nc.vector
                eng2 = nc.gpsimd if pi in adds_pool else nc.vector
                prio(eng1.tensor_tensor(
                    out=sl, in0=sq3[:, :, 0], in1=sq3[:, :, 1], op=mybir.AluOpType.add
                ), pi, 3)
                prio(eng2.tensor_tensor(
                    out=sl, in0=sl, in1=sq3[:, :, 2], op=mybir.AluOpType.add
                ), pi, 4)

            # stage B (per pair): ACT sqrt -> DVE masked accumulate
            def stage_b(pi, s2, M2):
                sz = str(M2)
                c0 = col_of[pi]
                pi0 = pi % n_pairs
                use_stt = stt and pi0 not in stt_skip
                if not use_stt:
                    # count(s > 1): only depends on s, runs while ACT sqrts.
                    # With bf16 s the all-bf16 operands put this TS in DVE 4x
                    # mode. (TensorScalarPtr is DVE-only on core V3 —
                    # count_pool fails walrus codegen; sim experiments only.)
                    ceng = nc.gpsimd if count_pool else nc.vector
                    cscr = scr16 if s2.dtype == mybir.dt.bfloat16 else scr32
                    acc_writers.append(prio(ceng.tensor_scalar(
                        out=cscr[:, :M2],
                        in0=s2,
                        scalar1=float(HALF_BODY * HALF_BODY),
                        scalar2=None,
                        op0=mybir.AluOpType.is_gt,
                        op1=mybir.AluOpType.add,
                        accum_out=accs[:, c0 + 1 : c0 + 2],
                    ), pi, 5))
                db = {"bufs": 2} if M2 >= 2176 else {}
                d = spool.tile([P, M2], mybir.dt.bfloat16, tag="d" + sz, **db)
                prio(nc.scalar.activation(
                    out=d, in_=s2, func=mybir.ActivationFunctionType.Sqrt
                ), pi, 6)
                if use_stt:
                    # one fused DVE op: accum += sum((s > 1) * d). The mask
                    # comes from fp32 s (exact), so bf16 d contributes only
                    # symmetric per-element rounding noise that cancels.
                    acc_writers.append(prio(nc.vector.scalar_tensor_tensor(
                        out=scr16[:, :M2],
                        in0=s2,
                        scalar=float(HALF_BODY * HALF_BODY),
                        in1=d,
                        op0=mybir.AluOpType.is_gt,
                        op1=mybir.AluOpType.mult,
                        accum_out=accs[:, c0 : c0 + 1],
                    ), pi, 7))
                else:
                    # sum(max(d, 1)) = masked_sum + M2 - count per partition;
                    # bf16 all-operands makes this TS 4x mode (cheap on the
                    # tail spine). Host subtracts P*M2 for these pairs.
                    acc_writers.append(prio(nc.vector.tensor_scalar(
                        out=scr16[:, :M2],
                        in0=d,
                        scalar1=float(HALF_BODY),
                        scalar2=None,
                        op0=mybir.AluOpType.max,
                        op1=mybir.AluOpType.add,
                        accum_out=accs[:, c0 : c0 + 1],
                    ), pi, 7))

            # emit: stage_a per tile; stage_b lags `lag` pairs behind.
            # repeat>1 re-runs the whole pass (benchmarking only).
            from collections import deque

            pending = deque()  # (pair_idx, s2, M2)
            for r in range(repeat):
                col0 = 0
                gi = 0
                for pi0, pair in enumerate(plan):
                    pi = r * n_pairs + pi0
                    # emit pending stage_b BEFORE this pair's stage_a, so a
                    # ready sqrt is not queued behind a DMA-gated square
                    if flush_before is not None and pi0 >= flush_before:
                        while pending:
                            stage_b(*pending.popleft())
                    M2 = sum(pair) // 3
                    sb = {"bufs": 2} if M2 >= 2176 else {}
                    # bf16 s for count-style pairs: count/max both hit DVE 4x.
                    # Cost: the bf16 band just above s=1 is excluded from both
                    # count and the d>1 region (~1e-4 systematic, gate 2e-2).
                    sdt = (mybir.dt.bfloat16
                           if (s_bf16 and stt and pi0 in stt_skip) else _DT)
                    stag = ("sb" if sdt == mybir.dt.bfloat16 else "s") + str(M2)
                    s2 = spool.tile([P, M2], sdt, tag=stag, **sb)
                    s_off = 0
                    for F in pair:
                        stage_a(gi, pi, col0, F, s2, s_off)
                        col0 += F
                        s_off += F // 3
                        gi += 1
                    if pipelined:
                        pending.append((pi, s2, M2))
                        # from pair `flush_from` on, emit stage_b immediately
                        # so tail sqrts outrank later squares in the scheduler
                        eff_lag = 0 if (flush_from is not None and pi0 >= flush_from) else lag
                        while len(pending) > eff_lag:
                            stage_b(*pending.popleft())
                    else:
                        stage_b(pi, s2, M2)
            while pending:
                stage_b(*pending.popleft())

            if not wb_out:
                nc.sync.dma_start(out=out[:, :], in_=accs)
                wb_prep = None
            else:
                # out-DMA via pre-staged SWDGE descriptors: the prep only
                # writes descriptors (no HWDGE/dge-delay on the tail), the
                # trigger fires the tiny transfer after the last accum.
                in_view = accs[:, :].rearrange("p (a b w) -> p a b w", a=1, b=1)
                out_view = out[:, :].rearrange("p (a b w) -> a p b w", a=1, b=1)
                wb_prep = nc.gpsimd.kv_writeback(
                    out_view, in_view, wb_idx[:, :], prepare_only=True,
                    sem=wb_sem,
                )
                wb_trig = nc.gpsimd.trigger_dma(count=None)
                # defer the RAW edges on accs from the prep to the trigger
                # (what bass_rust's swdge_deferred_ins does for scatter_add;
                # kv_writeback is not in that table)
                acc_names = {w.ins.name for w in acc_writers}
                prep_sync = list(wb_prep.ins.sync_dependency_names())
                wb_prep.ins.set_sync_dependencies(
                    NameSet([d for d in prep_sync if d not in acc_names])
                )
                # no nosync edges either: the prep writes descriptors only,
                # so it carries no ordering against the accum writers at all
                wb_prep.ins.set_nosync_dependencies(NameSet(
                    [d for d in wb_prep.ins.nosync_dependency_names()
                     if d not in acc_names]
                ))
                wb_trig.ins.set_sync_dependencies(NameSet(
                    list(wb_trig.ins.sync_dependency_names())
                    + sorted(acc_names)
                ))

    nc.compile()

    if wb_prep is not None:
        # the cost model fires the prep's on_update[0] at trigger time, and
        # the teardown drain waits on the framework's DMASW queue sem; point
        # on_update[0] at that sem (scatter_add gets this wiring natively)
        dmasw = None
        for i in nc.all_instructions():
            if i.sync_info:
                for w in i.sync_info.on_wait:
                    if w.ant_name and w.ant_name.startswith("DMASW"):
                        dmasw = (w.id, w.ant_name)
        assert dmasw is not None, "no DMASW drain wait found"
        wb_prep.ins.sync_info.on_update[0] = mybir.SyncUpdate(
            sync_type="semaphore", id=dmasw[0], ant_name=dmasw[1],
            update_mode="sem-add-imm", update_value=16,
        )
    return nc


_nc_cache = None
last_results = None


def kernel(kps_world_pred: np.ndarray) -> np.ndarray:
    global _nc_cache, last_results
    x = np.ascontiguousarray(kps_world_pred, dtype=np.float32)
    assert x.shape == (B, J, D)

    shards = x.reshape(N_CORES, P, COLS)
    in_maps = [{"x": shards[c]} for c in range(N_CORES)]

    if _nc_cache is None:
        _nc_cache = build_nc()

    # the axon terminal occasionally reports a transient
    # NRT_EXEC_UNIT_UNRECOVERABLE left over from a previous run; it clears
    # after a short wait, so retry rather than fail the whole call
    import time

    res = None
    for attempt in range(3):
        try:
            res = run_bass_kernel_spmd(_nc_cache, in_maps, list(range(N_CORES)))
            break
        except Exception:
            if attempt == 2:
                raise
            time.sleep(15)
    last_results = res

    # stt pairs contribute sum(d * (d > 1)) directly; stt_skip pairs
    # contribute sum(max(d,1)) + count = masked_sum + P*M2 per core
    total = np.float64(0.0)
    for c in range(N_CORES):
        total += res.results[c]["out"].astype(np.float64).sum()
    skip_m2 = sum(sum(PLAN[pi]) // 3 for pi in STT_SKIP)
    total -= np.float64(N_CORES * P * skip_m2)
    return np.asarray(total / (B * J), dtype=np.float32)



# revision 72
# speedup vs baseline: 1.0022x; 1.0004x over previous
"""Trainium2 Bass kernel for BodyStructureLoss.

Computes: mean over (B, J) of where(||kps[b,j,:]|| > 1.0, ||kps[b,j,:]||, 0)
for kps of shape [524288, 17, 3] float32.

Strategy (data-parallel over 8 NeuronCores):
  - Each core gets B/8 = 65536 batch rows = 3,342,336 contiguous floats,
    viewed as [128 partitions, 26112] (each partition row holds 8704
    complete (x,y,z) triplets).
  - Tiles of F columns stream in via DMA; squares run in-place (ACT, or DVE
    for a few tiles to balance engines); DVE sums the 3 squared components
    with two strided adds into a shared per-pair s tile; per tile PAIR one
    ACT sqrt and two DVE tensor_scalar+accumulate ops produce
    sum(max(d,1)) and count(s>1) columns.
  - Per core the [128, 2*n_pairs] accumulator tile is DMA'd out directly;
    the host sums all partials across cores and applies
    masked_sum = sum(max(d,1)) + count - B*J, then divides by B*J.
"""

import os

import numpy as np

# the NTFF trace path needs antenv.axon_hooks, which this client image lacks;
# force-disable so a stray BASS_TRACE=1 in the environment cannot break runs
os.environ["BASS_NEVER_TRACE"] = "1"

import concourse.bacc as bacc
import concourse.mybir as mybir
from concourse.bass_utils import run_bass_kernel_spmd
from concourse.tile import TileContext

B, J, D = 524288, 17, 3
HALF_BODY = 1.0  # threshold/2 with threshold=2.0
N_CORES = 8
B_SHARD = B // N_CORES  # 65536
P = 128
FLOATS_PER_CORE = B_SHARD * J * D  # 3342336
COLS = FLOATS_PER_CORE // P  # 26112 (divisible by 3: 26112 = 3*8704)

_DT = mybir.dt.float32

# default plan: pairs of tile column-counts; each pair shares one sqrt.
# small first pair ramps the compute pipeline early; small tail pairs
# shorten the post-DMA drain.
PLAN = [[456, 456]] + [[1632, 1632]] * 7 + [[768, 768], [360, 456]]
DVE_SQ = frozenset({9})  # tile indices squared on DVE instead of ACT
POOL_SQ = frozenset()  # tile indices squared on Pool (gpsimd)
# 1632-tiles whose square is split ACT (front 65%) / Pool (back 35%)
SPLIT_SQ = frozenset(range(2, 16))
# pairs that keep the count+max form (cheap 4x max on the tail spine);
# the host subtracts P*M2 per counted pair
STT_SKIP = frozenset({6, 7, 8, 9})


def build_nc(P=P, COLS=COLS, plan=None, dve_sq=DVE_SQ, pipelined=True, repeat=1, lag=2, flush_from=None, split_sq=SPLIT_SQ, pool_mode="stack", flush_before=None, pool_sq=POOL_SQ, count_pool=False, use_prio=False, wb_out=True, stt=True, add1_pool=frozenset(), stt_skip=STT_SKIP, xbufs=6, split_frac=0.65, adds_pool=frozenset({9}), s_bf16=False, split_frac_tail=None, tail_from=12):
    import bass_rust
    NameSet = bass_rust.InstructionNameOrderedSet
    if plan is None:
        plan = PLAN
    flat = [f for pair in plan for f in pair]
    assert sum(flat) == COLS
    assert all(f % 3 == 0 for f in flat)
    n_pairs = len(plan)
    M_MAX = max(sum(pair) for pair in plan) // 3

    nc = bacc.Bacc(
        "TRN2", target_bir_lowering=False, debug=False, num_devices=N_CORES
    )
    # stt pairs use one accumulator column; stt_skip pairs use two
    # (sum(max(d,1)) and count), requiring the host-side -M2*P correction
    col_of = {}
    off = 0
    for r in range(repeat):
        for pi0 in range(n_pairs):
            col_of[r * n_pairs + pi0] = off
            off += 2 if (not stt or pi0 in stt_skip) else 1
    acc_cols = off
    x = nc.dram_tensor("x", [P, COLS], _DT, kind="ExternalInput")
    out = nc.dram_tensor("out", [P, acc_cols], _DT, kind="ExternalOutput")

    with TileContext(nc, pool_alloc_mode=pool_mode) as tc:
        with (
            tc.tile_pool(name="xin", bufs=xbufs) as xpool,
            tc.tile_pool(name="small", bufs=4) as spool,
            tc.tile_pool(name="accp", bufs=1) as accpool,
        ):
            # stt: one accumulator column per pair, sum(d * (d > 1));
            # else two: sum(max(d,1)) and count(s>1)
            accs = accpool.tile([P, acc_cols], _DT)
            # shared scratch for tensor_scalar main outputs (only accum_out
            # is consumed); WAW chains are DVE-internal and in-order.
            # scr32 is only read by count pairs, so size it to their max M2.
            m32 = max([sum(plan[pi]) // 3 for pi in stt_skip] or [1]) if stt else M_MAX
            scr32 = accpool.tile([P, max(m32, 2)], _DT)
            scr16 = accpool.tile([P, M_MAX], mybir.dt.bfloat16)

            # dummy sqrt first: makes bacc's table pass load sqrt_and_others
            # (which also contains Square), avoiding a second ACT table load
            nc.vector.memset(scr32[:, :1], 1.0)
            nc.scalar.activation(
                out=scr32[:, :1],
                in_=scr32[:, :1],
                func=mybir.ActivationFunctionType.Sqrt,
            )

            acc_writers = []  # TS instructions writing accs (gate the out-DMA)
            if wb_out:
                wb_idx = accpool.tile([P, 1], mybir.dt.int32)
                nc.gpsimd.memset(wb_idx, 0)
                wb_sem = nc.alloc_semaphore("wb_dma")

            # scheduler priority: the tile scheduler pops the lowest
            # bass_priority READY instruction per engine; strict pair-order
            # priorities make a ready stage_b op always outrank later pairs'
            # squares on the same engine.
            def prio(ret, pi, off):
                if use_prio:
                    ret.ins.bass_priority = pi * 100 + off
                return ret

            # stage A (per tile): DMA -> square in place -> adds into s slice
            def stage_a(gi, pi, col0, F, s2, s_off):
                M = F // 3
                sz = str(F)
                nb = xbufs if F < 2448 else min(xbufs, 3)
                xb = {"bufs": nb} if nb != xbufs else {}
                xt = xpool.tile([P, F], _DT, tag="xt" + sz, **xb)
                prio(nc.sync.dma_start(out=xt, in_=x[:, col0 : col0 + F]), pi, 1)

                if gi in dve_sq:
                    r = nc.vector.tensor_tensor(
                        out=xt, in0=xt, in1=xt, op=mybir.AluOpType.mult
                    )
                elif gi in pool_sq:
                    r = nc.gpsimd.tensor_tensor(
                        out=xt, in0=xt, in1=xt, op=mybir.AluOpType.mult
                    )
                elif gi in split_sq:
                    # fractional engine split: ACT squares the front part,
                    # Pool (gpsimd) the back part of this tile. Chunks stay
                    # well under the pair cadence, so Pool cannot convoy.
                    f = split_frac if (split_frac_tail is None or gi < tail_from) else split_frac_tail
                    c = int(F * f) & ~3
                    prio(nc.scalar.activation(
                        out=xt[:, :c], in_=xt[:, :c],
                        func=mybir.ActivationFunctionType.Square,
                    ), pi, 2)
                    r = nc.gpsimd.tensor_tensor(
                        out=xt[:, c:], in0=xt[:, c:], in1=xt[:, c:],
                        op=mybir.AluOpType.mult,
                    )
                else:
                    r = nc.scalar.activation(
                        out=xt, in_=xt, func=mybir.ActivationFunctionType.Square
                    )
                prio(r, pi, 2)
                sq3 = xt.rearrange("p (m t) -> p m t", t=3)
                sl = s2[:, s_off : s_off + M]
                # adds_pool pairs run both adds on Pool: at the tail this
                # takes the s-computation off the (saturated) DVE chain
                eng1 = nc.gpsimd if pi in (add1_pool | adds_pool) else nc.vector
                eng2 = nc.gpsimd if pi in adds_pool else nc.vector
                prio(eng1.tensor_tensor(
                    out=sl, in0=sq3[:, :, 0], in1=sq3[:, :, 1], op=mybir.AluOpType.add
                ), pi, 3)
                prio(eng2.tensor_tensor(
                    out=sl, in0=sl, in1=sq3[:, :, 2], op=mybir.AluOpType.add
                ), pi, 4)

            # stage B (per pair): ACT sqrt -> DVE masked accumulate
            def stage_b(pi, s2, M2):
                sz = str(M2)
                c0 = col_of[pi]
                pi0 = pi % n_pairs
                use_stt = stt and pi0 not in stt_skip
                if not use_stt:
                    # count(s > 1): only depends on s, runs while ACT sqrts.
                    # With bf16 s the all-bf16 operands put this TS in DVE 4x
                    # mode. (TensorScalarPtr is DVE-only on core V3 —
                    # count_pool fails walrus codegen; sim experiments only.)
                    ceng = nc.gpsimd if count_pool else nc.vector
                    cscr = scr16 if s2.dtype == mybir.dt.bfloat16 else scr32
                    acc_writers.append(prio(ceng.tensor_scalar(
                        out=cscr[:, :M2],
                        in0=s2,
                        scalar1=float(HALF_BODY * HALF_BODY),
                        scalar2=None,
                        op0=mybir.AluOpType.is_gt,
                        op1=mybir.AluOpType.add,
                        accum_out=accs[:, c0 + 1 : c0 + 2],
                    ), pi, 5))
                db = {"bufs": 2} if M2 >= 2176 else {}
                d = spool.tile([P, M2], mybir.dt.bfloat16, tag="d" + sz, **db)
                prio(nc.scalar.activation(
                    out=d, in_=s2, func=mybir.ActivationFunctionType.Sqrt
                ), pi, 6)
                if use_stt:
                    # one fused DVE op: accum += sum((s > 1) * d). The mask
                    # comes from fp32 s (exact), so bf16 d contributes only
                    # symmetric per-element rounding noise that cancels.
                    acc_writers.append(prio(nc.vector.scalar_tensor_tensor(
                        out=scr16[:, :M2],
                        in0=s2,
                        scalar=float(HALF_BODY * HALF_BODY),
                        in1=d,
                        op0=mybir.AluOpType.is_gt,
                        op1=mybir.AluOpType.mult,
                        accum_out=accs[:, c0 : c0 + 1],
                    ), pi, 7))
                else:
                    # sum(max(d, 1)) = masked_sum + M2 - count per partition;
                    # bf16 all-operands makes this TS 4x mode (cheap on the
                    # tail spine). Host subtracts P*M2 for these pairs.
                    acc_writers.append(prio(nc.vector.tensor_scalar(
                        out=scr16[:, :M2],
                        in0=d,
                        scalar1=float(HALF_BODY),
                        scalar2=None,
                        op0=mybir.AluOpType.max,
                        op1=mybir.AluOpType.add,
                        accum_out=accs[:, c0 : c0 + 1],
                    ), pi, 7))

            # emit: stage_a per tile; stage_b lags `lag` pairs behind.
            # repeat>1 re-runs the whole pass (benchmarking only).
            from collections import deque

            pending = deque()  # (pair_idx, s2, M2)
            for r in range(repeat):
                col0 = 0
                gi = 0
                for pi0, pair in enumerate(plan):
                    pi = r * n_pairs + pi0
                    # emit pending stage_b BEFORE this pair's stage_a, so a
                    # ready sqrt is not queued behind a DMA-gated square
                    if flush_before is not None and pi0 >= flush_before:
                        while pending:
                            stage_b(*pending.popleft())
                    M2 = sum(pair) // 3
                    sb = {"bufs": 2} if M2 >= 2176 else {}
                    # bf16 s for count-style pairs: count/max both hit DVE 4x.
                    # Cost: the bf16 band just above s=1 is excluded from both
                    # count and the d>1 region (~1e-4 systematic, gate 2e-2).
                    sdt = (mybir.dt.bfloat16
                           if (s_bf16 and stt and pi0 in stt_skip) else _DT)
                    stag = ("sb" if sdt == mybir.dt.bfloat16 else "s") + str(M2)
                    s2 = spool.tile([P, M2], sdt, tag=stag, **sb)
                    s_off = 0
                    for F in pair:
                        stage_a(gi, pi, col0, F, s2, s_off)
                        col0 += F
                        s_off += F // 3
                        gi += 1
                    if pipelined:
                        pending.append((pi, s2, M2))
                        # from pair `flush_from` on, emit stage_b immediately
                        # so tail sqrts outrank later squares in the scheduler
                        eff_lag = 0 if (flush_from is not None and pi0 >= flush_from) else lag
                        while len(pending) > eff_lag:
                            stage_b(*pending.popleft())
                    else:
                        stage_b(pi, s2, M2)
            while pending:
                stage_b(*pending.popleft())

            if not wb_out:
                nc.sync.dma_start(out=out[:, :], in_=accs)
                wb_prep = None
            else:
                # out-DMA via pre-staged SWDGE descriptors: the prep only
                # writes descriptors (no HWDGE/dge-delay on the tail), the
                # trigger fires the tiny transfer after the last accum.
                in_view = accs[:, :].rearrange("p (a b w) -> p a b w", a=1, b=1)
                out_view = out[:, :].rearrange("p (a b w) -> a p b w", a=1, b=1)
                wb_prep = nc.gpsimd.kv_writeback(
                    out_view, in_view, wb_idx[:, :], prepare_only=True,
                    sem=wb_sem,
                )
                wb_trig = nc.gpsimd.trigger_dma(count=None)
                # defer the RAW edges on accs from the prep to the trigger
                # (what bass_rust's swdge_deferred_ins does for scatter_add;
                # kv_writeback is not in that table)
                acc_names = {w.ins.name for w in acc_writers}
                prep_sync = list(wb_prep.ins.sync_dependency_names())
                wb_prep.ins.set_sync_dependencies(
                    NameSet([d for d in prep_sync if d not in acc_names])
                )
                # no nosync edges either: the prep writes descriptors only,
                # so it carries no ordering against the accum writers at all
                wb_prep.ins.set_nosync_dependencies(NameSet(
                    [d for d in wb_prep.ins.nosync_dependency_names()
                     if d not in acc_names]
                ))
                wb_trig.ins.set_sync_dependencies(NameSet(
                    list(wb_trig.ins.sync_dependency_names())
                    + sorted(acc_names)
                ))

    nc.compile()

    if wb_prep is not None:
        # the cost model fires the prep's on_update[0] at trigger time, and
        # the teardown drain waits on the framework's DMASW queue sem; point
        # on_update[0] at that sem (scatter_add gets this wiring natively)
        dmasw = None
        for i in nc.all_instructions():
            if i.sync_info:
                for w in i.sync_info.on_wait:
                    if w.ant_name and w.ant_name.startswith("DMASW"):
                        dmasw = (w.id, w.ant_name)
        assert dmasw is not None, "no DMASW drain wait found"
        wb_prep.ins.sync_info.on_update[0] = mybir.SyncUpdate(
            sync_type="semaphore", id=dmasw[0], ant_name=dmasw[1],
            update_mode="sem-add-imm", update_value=16,
        )
    return nc


_nc_cache = None
last_results = None


def kernel(kps_world_pred: np.ndarray) -> np.ndarray:
    global _nc_cache, last_results
    x = np.ascontiguousarray(kps_world_pred, dtype=np.float32)
    assert x.shape == (B, J, D)

    shards = x.reshape(N_CORES, P, COLS)
    in_maps = [{"x": shards[c]} for c in range(N_CORES)]

    if _nc_cache is None:
        _nc_cache = build_nc()

    # the axon terminal occasionally reports a transient
    # NRT_EXEC_UNIT_UNRECOVERABLE left over from a previous run; it clears
    # after a short wait, so retry rather than fail the whole call
    import time

    res = None
    for attempt in range(3):
        try:
            res = run_bass_kernel_spmd(_nc_cache, in_maps, list(range(N_CORES)))
            break
        except Exception:
            if attempt == 2:
                raise
            time.sleep(15)
    last_results = res

    # stt pairs contribute sum(d * (d > 1)) directly; stt_skip pairs
    # contribute sum(max(d,1)) + count = masked_sum + P*M2 per core
    total = np.float64(0.0)
    for c in range(N_CORES):
        total += res.results[c]["out"].astype(np.float64).sum()
    skip_m2 = sum(sum(PLAN[pi]) // 3 for pi in STT_SKIP)
    total -= np.float64(N_CORES * P * skip_m2)
    return np.asarray(total / (B * J), dtype=np.float32)



# revision 73
# speedup vs baseline: 1.0024x; 1.0002x over previous
"""Trainium2 Bass kernel for BodyStructureLoss.

Computes: mean over (B, J) of where(||kps[b,j,:]|| > 1.0, ||kps[b,j,:]||, 0)
for kps of shape [524288, 17, 3] float32.

Strategy (data-parallel over 8 NeuronCores):
  - Each core gets B/8 = 65536 batch rows = 3,342,336 contiguous floats,
    viewed as [128 partitions, 26112] (each partition row holds 8704
    complete (x,y,z) triplets).
  - Tiles of F columns stream in via DMA; squares run in-place (ACT, or DVE
    for a few tiles to balance engines); DVE sums the 3 squared components
    with two strided adds into a shared per-pair s tile; per tile PAIR one
    ACT sqrt and two DVE tensor_scalar+accumulate ops produce
    sum(max(d,1)) and count(s>1) columns.
  - Per core the [128, 2*n_pairs] accumulator tile is DMA'd out directly;
    the host sums all partials across cores and applies
    masked_sum = sum(max(d,1)) + count - B*J, then divides by B*J.
"""

import os

import numpy as np

# the NTFF trace path needs antenv.axon_hooks, which this client image lacks;
# force-disable so a stray BASS_TRACE=1 in the environment cannot break runs
os.environ["BASS_NEVER_TRACE"] = "1"

import concourse.bacc as bacc
import concourse.mybir as mybir
from concourse.bass_utils import run_bass_kernel_spmd
from concourse.tile import TileContext

B, J, D = 524288, 17, 3
HALF_BODY = 1.0  # threshold/2 with threshold=2.0
N_CORES = 8
B_SHARD = B // N_CORES  # 65536
P = 128
FLOATS_PER_CORE = B_SHARD * J * D  # 3342336
COLS = FLOATS_PER_CORE // P  # 26112 (divisible by 3: 26112 = 3*8704)

_DT = mybir.dt.float32

# default plan: pairs of tile column-counts; each pair shares one sqrt.
# small first pair ramps the compute pipeline early; small tail pairs
# shorten the post-DMA drain.
PLAN = [[456, 456]] + [[1728, 1536]] * 7 + [[768, 768], [360, 456]]
DVE_SQ = frozenset({9})  # tile indices squared on DVE instead of ACT
POOL_SQ = frozenset()  # tile indices squared on Pool (gpsimd)
# 1632-tiles whose square is split ACT (front 65%) / Pool (back 35%)
SPLIT_SQ = frozenset(range(2, 16))
# pairs that keep the count+max form (cheap 4x max on the tail spine);
# the host subtracts P*M2 per counted pair
STT_SKIP = frozenset({6, 7, 8, 9})


def build_nc(P=P, COLS=COLS, plan=None, dve_sq=DVE_SQ, pipelined=True, repeat=1, lag=2, flush_from=None, split_sq=SPLIT_SQ, pool_mode="stack", flush_before=None, pool_sq=POOL_SQ, count_pool=False, use_prio=False, wb_out=True, stt=True, add1_pool=frozenset(), stt_skip=STT_SKIP, xbufs=6, split_frac=0.65, adds_pool=frozenset({9}), s_bf16=False, split_frac_tail=None, tail_from=12):
    import bass_rust
    NameSet = bass_rust.InstructionNameOrderedSet
    if plan is None:
        plan = PLAN
    flat = [f for pair in plan for f in pair]
    assert sum(flat) == COLS
    assert all(f % 3 == 0 for f in flat)
    n_pairs = len(plan)
    M_MAX = max(sum(pair) for pair in plan) // 3

    nc = bacc.Bacc(
        "TRN2", target_bir_lowering=False, debug=False, num_devices=N_CORES
    )
    # stt pairs use one accumulator column; stt_skip pairs use two
    # (sum(max(d,1)) and count), requiring the host-side -M2*P correction
    col_of = {}
    off = 0
    for r in range(repeat):
        for pi0 in range(n_pairs):
            col_of[r * n_pairs + pi0] = off
            off += 2 if (not stt or pi0 in stt_skip) else 1
    acc_cols = off
    x = nc.dram_tensor("x", [P, COLS], _DT, kind="ExternalInput")
    out = nc.dram_tensor("out", [P, acc_cols], _DT, kind="ExternalOutput")

    with TileContext(nc, pool_alloc_mode=pool_mode) as tc:
        with (
            tc.tile_pool(name="xin", bufs=xbufs) as xpool,
            tc.tile_pool(name="small", bufs=4) as spool,
            tc.tile_pool(name="accp", bufs=1) as accpool,
        ):
            # stt: one accumulator column per pair, sum(d * (d > 1));
            # else two: sum(max(d,1)) and count(s>1)
            accs = accpool.tile([P, acc_cols], _DT)
            # shared scratch for tensor_scalar main outputs (only accum_out
            # is consumed); WAW chains are DVE-internal and in-order.
            # scr32 is only read by count pairs, so size it to their max M2.
            m32 = max([sum(plan[pi]) // 3 for pi in stt_skip] or [1]) if stt else M_MAX
            scr32 = accpool.tile([P, max(m32, 2)], _DT)
            scr16 = accpool.tile([P, M_MAX], mybir.dt.bfloat16)

            # dummy sqrt first: makes bacc's table pass load sqrt_and_others
            # (which also contains Square), avoiding a second ACT table load
            nc.vector.memset(scr32[:, :1], 1.0)
            nc.scalar.activation(
                out=scr32[:, :1],
                in_=scr32[:, :1],
                func=mybir.ActivationFunctionType.Sqrt,
            )

            acc_writers = []  # TS instructions writing accs (gate the out-DMA)
            if wb_out:
                wb_idx = accpool.tile([P, 1], mybir.dt.int32)
                nc.gpsimd.memset(wb_idx, 0)
                wb_sem = nc.alloc_semaphore("wb_dma")

            # scheduler priority: the tile scheduler pops the lowest
            # bass_priority READY instruction per engine; strict pair-order
            # priorities make a ready stage_b op always outrank later pairs'
            # squares on the same engine.
            def prio(ret, pi, off):
                if use_prio:
                    ret.ins.bass_priority = pi * 100 + off
                return ret

            # stage A (per tile): DMA -> square in place -> adds into s slice
            def stage_a(gi, pi, col0, F, s2, s_off):
                M = F // 3
                sz = str(F)
                nb = xbufs if F < 2448 else min(xbufs, 3)
                xb = {"bufs": nb} if nb != xbufs else {}
                xt = xpool.tile([P, F], _DT, tag="xt" + sz, **xb)
                prio(nc.sync.dma_start(out=xt, in_=x[:, col0 : col0 + F]), pi, 1)

                if gi in dve_sq:
                    r = nc.vector.tensor_tensor(
                        out=xt, in0=xt, in1=xt, op=mybir.AluOpType.mult
                    )
                elif gi in pool_sq:
                    r = nc.gpsimd.tensor_tensor(
                        out=xt, in0=xt, in1=xt, op=mybir.AluOpType.mult
                    )
                elif gi in split_sq:
                    # fractional engine split: ACT squares the front part,
                    # Pool (gpsimd) the back part of this tile. Chunks stay
                    # well under the pair cadence, so Pool cannot convoy.
                    f = split_frac if (split_frac_tail is None or gi < tail_from) else split_frac_tail
                    c = int(F * f) & ~3
                    prio(nc.scalar.activation(
                        out=xt[:, :c], in_=xt[:, :c],
                        func=mybir.ActivationFunctionType.Square,
                    ), pi, 2)
                    r = nc.gpsimd.tensor_tensor(
                        out=xt[:, c:], in0=xt[:, c:], in1=xt[:, c:],
                        op=mybir.AluOpType.mult,
                    )
                else:
                    r = nc.scalar.activation(
                        out=xt, in_=xt, func=mybir.ActivationFunctionType.Square
                    )
                prio(r, pi, 2)
                sq3 = xt.rearrange("p (m t) -> p m t", t=3)
                sl = s2[:, s_off : s_off + M]
                # adds_pool pairs run both adds on Pool: at the tail this
                # takes the s-computation off the (saturated) DVE chain
                eng1 = nc.gpsimd if pi in (add1_pool | adds_pool) else nc.vector
                eng2 = nc.gpsimd if pi in adds_pool else nc.vector
                prio(eng1.tensor_tensor(
                    out=sl, in0=sq3[:, :, 0], in1=sq3[:, :, 1], op=mybir.AluOpType.add
                ), pi, 3)
                prio(eng2.tensor_tensor(
                    out=sl, in0=sl, in1=sq3[:, :, 2], op=mybir.AluOpType.add
                ), pi, 4)

            # stage B (per pair): ACT sqrt -> DVE masked accumulate
            def stage_b(pi, s2, M2):
                sz = str(M2)
                c0 = col_of[pi]
                pi0 = pi % n_pairs
                use_stt = stt and pi0 not in stt_skip
                if not use_stt:
                    # count(s > 1): only depends on s, runs while ACT sqrts.
                    # With bf16 s the all-bf16 operands put this TS in DVE 4x
                    # mode. (TensorScalarPtr is DVE-only on core V3 —
                    # count_pool fails walrus codegen; sim experiments only.)
                    ceng = nc.gpsimd if count_pool else nc.vector
                    cscr = scr16 if s2.dtype == mybir.dt.bfloat16 else scr32
                    acc_writers.append(prio(ceng.tensor_scalar(
                        out=cscr[:, :M2],
                        in0=s2,
                        scalar1=float(HALF_BODY * HALF_BODY),
                        scalar2=None,
                        op0=mybir.AluOpType.is_gt,
                        op1=mybir.AluOpType.add,
                        accum_out=accs[:, c0 + 1 : c0 + 2],
                    ), pi, 5))
                db = {"bufs": 2} if M2 >= 2176 else {}
                d = spool.tile([P, M2], mybir.dt.bfloat16, tag="d" + sz, **db)
                prio(nc.scalar.activation(
                    out=d, in_=s2, func=mybir.ActivationFunctionType.Sqrt
                ), pi, 6)
                if use_stt:
                    # one fused DVE op: accum += sum((s > 1) * d). The mask
                    # comes from fp32 s (exact), so bf16 d contributes only
                    # symmetric per-element rounding noise that cancels.
                    acc_writers.append(prio(nc.vector.scalar_tensor_tensor(
                        out=scr16[:, :M2],
                        in0=s2,
                        scalar=float(HALF_BODY * HALF_BODY),
                        in1=d,
                        op0=mybir.AluOpType.is_gt,
                        op1=mybir.AluOpType.mult,
                        accum_out=accs[:, c0 : c0 + 1],
                    ), pi, 7))
                else:
                    # sum(max(d, 1)) = masked_sum + M2 - count per partition;
                    # bf16 all-operands makes this TS 4x mode (cheap on the
                    # tail spine). Host subtracts P*M2 for these pairs.
                    acc_writers.append(prio(nc.vector.tensor_scalar(
                        out=scr16[:, :M2],
                        in0=d,
                        scalar1=float(HALF_BODY),
                        scalar2=None,
                        op0=mybir.AluOpType.max,
                        op1=mybir.AluOpType.add,
                        accum_out=accs[:, c0 : c0 + 1],
                    ), pi, 7))

            # emit: stage_a per tile; stage_b lags `lag` pairs behind.
            # repeat>1 re-runs the whole pass (benchmarking only).
            from collections import deque

            pending = deque()  # (pair_idx, s2, M2)
            for r in range(repeat):
                col0 = 0
                gi = 0
                for pi0, pair in enumerate(plan):
                    pi = r * n_pairs + pi0
                    # emit pending stage_b BEFORE this pair's stage_a, so a
                    # ready sqrt is not queued behind a DMA-gated square
                    if flush_before is not None and pi0 >= flush_before:
                        while pending:
                            stage_b(*pending.popleft())
                    M2 = sum(pair) // 3
                    sb = {"bufs": 2} if M2 >= 2176 else {}
                    # bf16 s for count-style pairs: count/max both hit DVE 4x.
                    # Cost: the bf16 band just above s=1 is excluded from both
                    # count and the d>1 region (~1e-4 systematic, gate 2e-2).
                    sdt = (mybir.dt.bfloat16
                           if (s_bf16 and stt and pi0 in stt_skip) else _DT)
                    stag = ("sb" if sdt == mybir.dt.bfloat16 else "s") + str(M2)
                    s2 = spool.tile([P, M2], sdt, tag=stag, **sb)
                    s_off = 0
                    for F in pair:
                        stage_a(gi, pi, col0, F, s2, s_off)
                        col0 += F
                        s_off += F // 3
                        gi += 1
                    if pipelined:
                        pending.append((pi, s2, M2))
                        # from pair `flush_from` on, emit stage_b immediately
                        # so tail sqrts outrank later squares in the scheduler
                        eff_lag = 0 if (flush_from is not None and pi0 >= flush_from) else lag
                        while len(pending) > eff_lag:
                            stage_b(*pending.popleft())
                    else:
                        stage_b(pi, s2, M2)
            while pending:
                stage_b(*pending.popleft())

            if not wb_out:
                nc.sync.dma_start(out=out[:, :], in_=accs)
                wb_prep = None
            else:
                # out-DMA via pre-staged SWDGE descriptors: the prep only
                # writes descriptors (no HWDGE/dge-delay on the tail), the
                # trigger fires the tiny transfer after the last accum.
                in_view = accs[:, :].rearrange("p (a b w) -> p a b w", a=1, b=1)
                out_view = out[:, :].rearrange("p (a b w) -> a p b w", a=1, b=1)
                wb_prep = nc.gpsimd.kv_writeback(
                    out_view, in_view, wb_idx[:, :], prepare_only=True,
                    sem=wb_sem,
                )
                wb_trig = nc.gpsimd.trigger_dma(count=None)
                # defer the RAW edges on accs from the prep to the trigger
                # (what bass_rust's swdge_deferred_ins does for scatter_add;
                # kv_writeback is not in that table)
                acc_names = {w.ins.name for w in acc_writers}
                prep_sync = list(wb_prep.ins.sync_dependency_names())
                wb_prep.ins.set_sync_dependencies(
                    NameSet([d for d in prep_sync if d not in acc_names])
                )
                # no nosync edges either: the prep writes descriptors only,
                # so it carries no ordering against the accum writers at all
                wb_prep.ins.set_nosync_dependencies(NameSet(
                    [d for d in wb_prep.ins.nosync_dependency_names()
                     if d not in acc_names]
                ))
                wb_trig.ins.set_sync_dependencies(NameSet(
                    list(wb_trig.ins.sync_dependency_names())
                    + sorted(acc_names)
                ))

    nc.compile()

    if wb_prep is not None:
        # the cost model fires the prep's on_update[0] at trigger time, and
        # the teardown drain waits on the framework's DMASW queue sem; point
        # on_update[0] at that sem (scatter_add gets this wiring natively)
        dmasw = None
        for i in nc.all_instructions():
            if i.sync_info:
                for w in i.sync_info.on_wait:
                    if w.ant_name and w.ant_name.startswith("DMASW"):
                        dmasw = (w.id, w.ant_name)
        assert dmasw is not None, "no DMASW drain wait found"
        wb_prep.ins.sync_info.on_update[0] = mybir.SyncUpdate(
            sync_type="semaphore", id=dmasw[0], ant_name=dmasw[1],
            update_mode="sem-add-imm", update_value=16,
        )
    return nc


_nc_cache = None
last_results = None


def kernel(kps_world_pred: np.ndarray) -> np.ndarray:
    global _nc_cache, last_results
    x = np.ascontiguousarray(kps_world_pred, dtype=np.float32)
    assert x.shape == (B, J, D)

    shards = x.reshape(N_CORES, P, COLS)
    in_maps = [{"x": shards[c]} for c in range(N_CORES)]

    if _nc_cache is None:
        _nc_cache = build_nc()

    # the axon terminal occasionally reports a transient
    # NRT_EXEC_UNIT_UNRECOVERABLE left over from a previous run; it clears
    # after a short wait, so retry rather than fail the whole call
    import time

    res = None
    for attempt in range(3):
        try:
            res = run_bass_kernel_spmd(_nc_cache, in_maps, list(range(N_CORES)))
            break
        except Exception:
            if attempt == 2:
                raise
            time.sleep(15)
    last_results = res

    # stt pairs contribute sum(d * (d > 1)) directly; stt_skip pairs
    # contribute sum(max(d,1)) + count = masked_sum + P*M2 per core
    total = np.float64(0.0)
    for c in range(N_CORES):
        total += res.results[c]["out"].astype(np.float64).sum()
    skip_m2 = sum(sum(PLAN[pi]) // 3 for pi in STT_SKIP)
    total -= np.float64(N_CORES * P * skip_m2)
    return np.asarray(total / (B * J), dtype=np.float32)

